# revision 19
# baseline (speedup 1.0000x reference)
"""Trainium2 Bass kernel for nn_CSA_36971078484033.

Instance-norm over (H,W) per (B,C) with a Dirichlet-weighted prototype affine
(label-conditional bank selection), data-parallel over B on 8 NeuronCores.

  out[b,c,h,w] = (x[b,c,h,w] - mean[b,c]) / sqrt(var[b,c] + eps) * new_std[b,c]
               + new_mean[b,c]
  new_mean = (label==0) ? w@proto_mean_pos : w@proto_mean_neg   (same for std)

Per core: 4 samples = 8 tiles of [128ch, 3136px].  Stats via bn_stats/bn_aggr
(DVE), affine apply via one ScalarE activation (out = x*scale + bias), the tiny
[64,4]x[64,256] prototype einsum on TensorE with the label selection folded
into host-masked weights (w*(label==0) and w*(label!=0) contribute to pos/neg
banks respectively; the unselected bank's weights are zero).

x/y travel as fp16 (host casts): per-core HBM traffic drops 25.7MB -> 12.8MB,
which is the binding roofline (~358 GB/s HBM per NC).  fp16 keeps 11 ktmantissa
bits: abs err ~5e-4 * |x|, orders below the 2e-2 gate.  Stats accumulate in
f32 inside DVE; ScalarE applies the f32 per-(b,c) affine with an fp16 cast on
the way out.
"""

import numpy as np
from contextlib import ExitStack

B, C, H, W = 32, 256, 56, 56
HW = H * W            # 3136
K = 64
EPS = 1e-5
NCORES = 8
BPC = B // NCORES     # 4 samples per core
ROWS = BPC * C        # 1024 DRAM rows per core
NCHUNK = 7
PCOLS = 4 + 2 * 256   # [wposT;wnegT] | [pmp;pmn] | [psp;psn], 128 rows
CHUNK = HW // NCHUNK  # 448 (<= bn_stats hw max of 512; equal chunks keep
                      # bn_aggr's equal-count variance combine exact)

_cache = {}


def _emit(tc, nc, mybir, aps):
    f32 = mybir.dt.float32
    f16 = mybir.dt.float16
    x_d, packed_d, y_d = aps
    with ExitStack() as ctx:
        consts = ctx.enter_context(tc.tile_pool(name="consts", bufs=1))
        xpool = ctx.enter_context(tc.tile_pool(name="xp", bufs=8))
        ypool = ctx.enter_context(tc.tile_pool(name="yp", bufs=4))
        stats = ctx.enter_context(tc.tile_pool(name="stats", bufs=4))
        psum = ctx.enter_context(tc.tile_pool(name="psum", bufs=2, space="PSUM"))

        # Scheduling: the Tile list-scheduler reorders per-engine streams
        # using its own sim; left alone it bunches the per-tile chain ops at
        # the end of the vector stream, which stalls ScalarE and serializes
        # a ~12us tail.  tile_wait_until(g) with a monotonically increasing
        # group index pins every engine's static order to exactly the
        # software pipeline below.
        gctr = [0]

        def grp(adv=True):
            w = tc.tile_wait_until(gctr[0])
            if adv:
                gctr[0] += 1
            return w

        # --- tiny inputs packed host-side into ONE [128, 516] tensor:
        # col 0:4   = [wposT; wnegT]  (label-masked Dirichlet weights, stacked
        #             pos-bank over neg-bank along the 128-partition dim)
        # col 4:260 = [pmp; pmn], col 260:516 = [psp; psn]
        # Dispatched on the Sync ring AHEAD of the x tiles (the Activation
        # ring starts with ~2.6us of ACT_TABLE_LOADs that would delay it).
        ntiles = BPC * 2
        xts = []
        packed_sb = consts.tile([2 * K, PCOLS], f32, tag="packed")
        eps_sb = consts.tile([128, 1], f32, tag="eps")
        with grp():
            nc.sync.dma_start(packed_sb[:], packed_d[:])
            nc.vector.memset(eps_sb[:], EPS)
            for ti in range(ntiles):
                b, h = divmod(ti, 2)
                r0 = b * C + h * 128
                x_sb = xpool.tile([128, HW], f16, tag="xt")
                nc.sync.dma_start(x_sb[:], x_d[r0:r0 + 128, :])
                xts.append((x_sb, r0, h * BPC + b))
        w_sb = packed_sb[:, 0:BPC]
        pmean = packed_sb[:, BPC:BPC + C]
        pstd = packed_sb[:, BPC + C:BPC + 2 * C]

        # selected new_mean/new_std, channel-major: [128ch, BPC] per half;
        # ONE 128-contraction matmul per (stat, chalf).  Runs during the
        # first x tile's in-DMA.
        mean_sel = consts.tile([128, 2 * BPC], f32, tag="mean_sel")
        std_sel = consts.tile([128, 2 * BPC], f32, tag="std_sel")
        with grp():
            for h in range(2):
                cs = slice(h * 128, (h + 1) * 128)
                bs = slice(h * BPC, (h + 1) * BPC)
                pm = psum.tile([128, BPC], f32, tag="ps_mm")
                nc.tensor.matmul(pm[:], pmean[:, cs], w_sb, start=True, stop=True)
                nc.vector.tensor_copy(mean_sel[:, bs], pm[:])
                ps = psum.tile([128, BPC], f32, tag="ps_ss")
                nc.tensor.matmul(ps[:], pstd[:, cs], w_sb, start=True, stop=True)
                nc.vector.tensor_copy(std_sel[:, bs], ps[:])

        # --- 8 tiles of [128, HW], software-pipelined.  Steady-state order:
        #   vector: [sq_i] [chain_{i-1}] [sum_i, varfix_i]
        #   scalar: [ID_{i-1}a, ID_{i-1}b] [sqrt_i]
        # Stats via two fp16 DVE passes (STT x*x with accum_out -> sum(x^2),
        # tensor_reduce -> sum(x)): fp16 elementwise DVE runs ~2.3x faster
        # than bn_stats' Welford path.  varn_neg = sum*mean - sumsq feeds the
        # ScalarE Sqrt with a NEGATIVE scale (-1/(N-1)) to fold the sign.
        ntiles = BPC * 2
        mvs, affs = [], []
        scratch = ctx.enter_context(tc.tile_pool(name="sc", bufs=2))

        def emit_stats_head(ti):
            x_sb, _, _ = xts[ti]
            sq = scratch.tile([128, HW], f16, tag="sq")
            ssq = stats.tile([128, 1], f32, tag="ssq")
            with grp():
                nc.vector.scalar_tensor_tensor(sq[:], x_sb[:], 1.0, x_sb[:],
                                               mybir.AluOpType.mult,
                                               mybir.AluOpType.mult,
                                               accum_out=ssq[:])
            return ssq

        def emit_stats_tail(ti, ssq):
            x_sb, _, _ = xts[ti]
            with grp():
                ssum = stats.tile([128, 1], f32, tag="ssum")
                nc.vector.tensor_reduce(ssum[:], x_sb[:],
                                        mybir.AxisListType.X,
                                        mybir.AluOpType.add)
                mean = stats.tile([128, 1], f32, tag="mean")
                nc.vector.tensor_scalar(mean[:], ssum[:], 1.0 / HW, None,
                                        mybir.AluOpType.mult)
                varn_neg = stats.tile([128, 1], f32, tag="varn")
                nc.vector.scalar_tensor_tensor(varn_neg[:], ssum[:], mean[:],
                                               ssq[:],
                                               mybir.AluOpType.mult,
                                               mybir.AluOpType.subtract)
                # std = sqrt(varn/(N-1) + eps) on ScalarE; varn_neg's sign is
                # absorbed by the negative scale
                stdv = stats.tile([128, 1], f32, tag="stdv")
                nc.scalar.activation(stdv[:], varn_neg[:],
                                     mybir.ActivationFunctionType.Sqrt,
                                     bias=eps_sb[:],
                                     scale=-1.0 / float(HW - 1))
            mvs.append((mean, stdv))

        def emit_chain(ti):
            mean, stdv = mvs[ti]
            col = xts[ti][2]
            with grp():
                rstd = stats.tile([128, 1], f32, tag="rstd")
                nc.vector.reciprocal(rstd[:], stdv[:])
                scl = stats.tile([128, 1], f32, tag="scl")
                nc.vector.tensor_mul(scl[:], rstd[:], std_sel[:, col:col + 1])
                tmp = stats.tile([128, 1], f32, tag="tmp")
                nc.vector.tensor_mul(tmp[:], mean[:], scl[:])
                shf = stats.tile([128, 1], f32, tag="shf")
                nc.vector.tensor_sub(shf[:], mean_sel[:, col:col + 1], tmp[:])
            affs.append((scl, shf))

        def emit_apply(ti):
            x_sb, r0, _ = xts[ti]
            scl, shf = affs[ti]
            y_sb = ypool.tile([128, HW], f16, tag="yt")
            if ti < ntiles - 1:
                # halves: the out-DMA of half 0 overlaps the IDENTITY of
                # half 1
                with grp():
                    for hs in (slice(0, HW // 2), slice(HW // 2, HW)):
                        nc.scalar.activation(
                            y_sb[:, hs], x_sb[:, hs],
                            mybir.ActivationFunctionType.Identity,
                            bias=shf[:], scale=scl[:])
                        # out-DMAs ride the Activation HWDGE ring: the Sync
                        # ring is FIFO, an out waiting on compute blocks ins
                        nc.scalar.dma_start(y_d[r0:r0 + 128, hs], y_sb[:, hs])
            else:
                # last tile: nothing left for the vector engine to do, so
                # split the apply across ScalarE and DVE to halve the drain
                # (DVE half's store rides the now-idle Sync ring)
                h0 = slice(0, HW // 2)
                h1 = slice(HW // 2, HW)
                with grp():
                    nc.vector.tensor_scalar(y_sb[:, h1], x_sb[:, h1],
                                            scl[:], shf[:],
                                            mybir.AluOpType.mult,
                                            mybir.AluOpType.add)
                    nc.sync.dma_start(y_d[r0:r0 + 128, h1], y_sb[:, h1])
                    nc.scalar.activation(
                        y_sb[:, h0], x_sb[:, h0],
                        mybir.ActivationFunctionType.Identity,
                        bias=shf[:], scale=scl[:])
                    nc.scalar.dma_start(y_d[r0:r0 + 128, h0], y_sb[:, h0])

        for ti in range(ntiles):
            st6 = emit_stats_head(ti)
            if ti > 0:
                emit_chain(ti - 1)
                emit_apply(ti - 1)
            emit_stats_tail(ti, st6)
        emit_chain(ntiles - 1)
        emit_apply(ntiles - 1)


def _program():
    if "nc" in _cache:
        return _cache["nc"]
    import concourse.bass as bass  # noqa: F401
    import concourse.tile as tile
    from concourse import bacc, mybir

    f32 = mybir.dt.float32
    f16 = mybir.dt.float16
    nc = bacc.Bacc("TRN2", target_bir_lowering=False, debug=False,
                   num_devices=NCORES)
    aps = [
        nc.dram_tensor("x", [ROWS, HW], f16, kind="ExternalInput").ap(),
        nc.dram_tensor("packed", [2 * K, PCOLS], f32, kind="ExternalInput").ap(),
        nc.dram_tensor("y", [ROWS, HW], f16, kind="ExternalOutput").ap(),
    ]
    with tile.TileContext(nc) as tc:
        _emit(tc, nc, mybir, aps)
    nc.compile()
    _cache["nc"] = nc
    return nc


def _run(inputs, trace=False, trace_cores=None):
    from concourse import bass_utils

    nc = _program()

    x = np.asarray(inputs["x"], dtype=np.float32)
    label = np.asarray(inputs["label"])
    w = np.asarray(inputs["combine_weights"], dtype=np.float32)
    pmp = np.ascontiguousarray(np.asarray(inputs["proto_mean_pos"], dtype=np.float32))
    psp = np.ascontiguousarray(np.asarray(inputs["proto_std_pos"], dtype=np.float32))
    pmn = np.ascontiguousarray(np.asarray(inputs["proto_mean_neg"], dtype=np.float32))
    psn = np.ascontiguousarray(np.asarray(inputs["proto_std_neg"], dtype=np.float32))

    is_pos = (label == 0).astype(np.float32)[:, None]   # [B,1]
    wpos = w * is_pos                                   # [B,K]
    wneg = w * (1.0 - is_pos)

    in_maps = []
    for c in range(NCORES):
        bs = slice(c * BPC, (c + 1) * BPC)
        packed = np.concatenate([
            np.concatenate([wpos[bs].T, wneg[bs].T], axis=0),
            np.concatenate([pmp, pmn], axis=0),
            np.concatenate([psp, psn], axis=0),
        ], axis=1)
        in_maps.append({
            "x": np.ascontiguousarray(x[bs]).reshape(ROWS, HW).astype(np.float16),
            "packed": np.ascontiguousarray(packed),
        })

    res = bass_utils.run_bass_kernel_spmd(
        nc, in_maps, core_ids=list(range(NCORES)),
        trace=trace, trace_cores=trace_cores,
    )
    out = np.concatenate(
        [np.asarray(res.results[c]["y"], dtype=np.float32).reshape(BPC, C, H, W)
         for c in range(NCORES)],
        axis=0,
    )
    return out, res


def kernel(**inputs):
    out, _ = _run(inputs, trace=False)
    return out



# revision 20
# speedup vs baseline: 1.4118x; 1.4118x over previous
"""Trainium2 Bass kernel for nn_CSA_36971078484033.

Instance-norm over (H,W) per (B,C) with a Dirichlet-weighted prototype affine
(label-conditional bank selection), data-parallel over B on 8 NeuronCores.

  out[b,c,h,w] = (x[b,c,h,w] - mean[b,c]) / sqrt(var[b,c] + eps) * new_std[b,c]
               + new_mean[b,c]
  new_mean = (label==0) ? w@proto_mean_pos : w@proto_mean_neg   (same for std)

Per core: 4 samples = 8 tiles of [128ch, 3136px].  Stats via bn_stats/bn_aggr
(DVE), affine apply via one ScalarE activation (out = x*scale + bias), the tiny
[64,4]x[64,256] prototype einsum on TensorE with the label selection folded
into host-masked weights (w*(label==0) and w*(label!=0) contribute to pos/neg
banks respectively; the unselected bank's weights are zero).

x/y travel as fp16 (host casts): per-core HBM traffic drops 25.7MB -> 12.8MB,
which is the binding roofline (~358 GB/s HBM per NC).  fp16 keeps 11 ktmantissa
bits: abs err ~5e-4 * |x|, orders below the 2e-2 gate.  Stats accumulate in
f32 inside DVE; ScalarE applies the f32 per-(b,c) affine with an fp16 cast on
the way out.
"""

import numpy as np
from contextlib import ExitStack

B, C, H, W = 32, 256, 56, 56
HW = H * W            # 3136
K = 64
EPS = 1e-5
NCORES = 8
BPC = B // NCORES     # 4 samples per core
ROWS = BPC * C        # 1024 DRAM rows per core
NCHUNK = 7
PCOLS = 4 + 2 * 256   # [wposT;wnegT] | [pmp;pmn] | [psp;psn], 128 rows
CHUNK = HW // NCHUNK  # 448 (<= bn_stats hw max of 512; equal chunks keep
                      # bn_aggr's equal-count variance combine exact)

_cache = {}


def _emit(tc, nc, mybir, aps):
    f32 = mybir.dt.float32
    f16 = mybir.dt.float16
    x_d, packed_d, y_d = aps
    with ExitStack() as ctx:
        consts = ctx.enter_context(tc.tile_pool(name="consts", bufs=1))
        xpool = ctx.enter_context(tc.tile_pool(name="xp", bufs=8))
        ypool = ctx.enter_context(tc.tile_pool(name="yp", bufs=4))
        stats = ctx.enter_context(tc.tile_pool(name="stats", bufs=4))
        psum = ctx.enter_context(tc.tile_pool(name="psum", bufs=2, space="PSUM"))

        # Scheduling: the Tile list-scheduler reorders per-engine streams
        # using its own sim; left alone it bunches the per-tile chain ops at
        # the end of the vector stream, which stalls ScalarE and serializes
        # a ~12us tail.  tile_wait_until(g) with a monotonically increasing
        # group index pins every engine's static order to exactly the
        # software pipeline below.
        gctr = [0]

        def grp(adv=True):
            w = tc.tile_wait_until(gctr[0])
            if adv:
                gctr[0] += 1
            return w

        # --- tiny inputs packed host-side into ONE [128, 516] tensor:
        # col 0:4   = [wposT; wnegT]  (label-masked Dirichlet weights, stacked
        #             pos-bank over neg-bank along the 128-partition dim)
        # col 4:260 = [pmp; pmn], col 260:516 = [psp; psn]
        # Dispatched on the Sync ring AHEAD of the x tiles (the Activation
        # ring starts with ~2.6us of ACT_TABLE_LOADs that would delay it).
        ntiles = BPC * 2
        NHEAD = 3
        xts = []
        packed_sb = consts.tile([2 * K, PCOLS], f32, tag="packed")
        eps_sb = consts.tile([128, 1], f32, tag="eps")
        with grp():
            nc.sync.dma_start(packed_sb[:], packed_d[:])
            nc.vector.memset(eps_sb[:], EPS)
            for ti in range(ntiles):
                b, h = divmod(ti, 2)
                r0 = b * C + h * 128
                x_sb = xpool.tile([128, HW], f16, tag="xt")
                if ti == 0:
                    # split tile 0 so bn_stats can start on the first NHEAD
                    # chunks ~1.5us before the whole tile lands
                    c0 = NHEAD * CHUNK
                    nc.sync.dma_start(x_sb[:, 0:c0], x_d[r0:r0 + 128, 0:c0])
                    nc.sync.dma_start(x_sb[:, c0:], x_d[r0:r0 + 128, c0:])
                else:
                    nc.sync.dma_start(x_sb[:], x_d[r0:r0 + 128, :])
                xts.append((x_sb, r0, h * BPC + b))
        w_sb = packed_sb[:, 0:BPC]
        pmean = packed_sb[:, BPC:BPC + C]
        pstd = packed_sb[:, BPC + C:BPC + 2 * C]
        mean_sel = consts.tile([128, 2 * BPC], f32, tag="mean_sel")
        std_sel = consts.tile([128, 2 * BPC], f32, tag="std_sel")

        def emit_protos():
            # selected new_mean/new_std, channel-major: [128ch, BPC] per
            # half; ONE 128-contraction matmul per (stat, chalf).  Runs
            # during the first x tile's in-DMA.
            with grp():
                for h in range(2):
                    cs = slice(h * 128, (h + 1) * 128)
                    bs = slice(h * BPC, (h + 1) * BPC)
                    pm = psum.tile([128, BPC], f32, tag="ps_mm")
                    nc.tensor.matmul(pm[:], pmean[:, cs], w_sb,
                                     start=True, stop=True)
                    nc.vector.tensor_copy(mean_sel[:, bs], pm[:])
                    ps = psum.tile([128, BPC], f32, tag="ps_ss")
                    nc.tensor.matmul(ps[:], pstd[:, cs], w_sb,
                                     start=True, stop=True)
                    nc.vector.tensor_copy(std_sel[:, bs], ps[:])

        # --- 8 tiles of [128, HW] processed as 4 PAIRS (tiles 2p, 2p+1 =
        # sample b, channel halves 0/1).  Stats per tile via bn_stats/
        # bn_aggr; the sqrt + affine chain is batched per pair as [128,2]
        # ops (strided APs over the pair's mv columns), which halves both
        # ScalarE's Sqrt<->Identity table-switch tax (~1.27us/switch-pair)
        # and the DVE small-op dispatch overhead (~160ns/op).
        # Steady-state emission per pair p (tiles t0=2p, t1=2p+1):
        #   [BN head t0] [chain p-1] [ID(2p-2)] [BN tail t0 + aggr]
        #   [BN head t1] [ID(2p-1)] [BN tail t1 + aggr] [sqrt pair p]
        mvps, affs = [], []

        def emit_bn(ti, mvp, mcol, nhead=0):
            x_sb, _, _ = xts[ti]
            st6 = stats.tile([128, NCHUNK * 6], f32, tag="st6")
            def run(lo, hi):
                for i in range(lo, hi):
                    nc.vector.bn_stats(st6[:, i * 6:(i + 1) * 6],
                                       x_sb[:, i * CHUNK:(i + 1) * CHUNK])
            if nhead:
                with grp():
                    run(0, nhead)
                yield
            with grp():
                run(nhead, NCHUNK)
                nc.vector.bn_aggr(mvp[:, 2 * mcol:2 * mcol + 2], st6[:])

        def emit_sqrt(p, mvp):
            # one Sqrt over both tiles' variances: [128,2] strided view
            stdv = stats.tile([128, 2], f32, tag="stdv")
            with grp():
                nc.scalar.activation(stdv[:], mvp[:, 1:4:2],
                                     mybir.ActivationFunctionType.Sqrt,
                                     bias=eps_sb[:],
                                     scale=float(HW) / float(HW - 1))
            mvps.append((mvp, stdv))

        def emit_chain(p):
            mvp, stdv = mvps[p]
            b = p  # pair p = sample b, cols b and BPC+b in *_sel
            cs = slice(b, 2 * BPC, BPC)
            with grp():
                rstd = stats.tile([128, 2], f32, tag="rstd")
                nc.vector.reciprocal(rstd[:], stdv[:])
                scl = stats.tile([128, 2], f32, tag="scl")
                nc.vector.tensor_mul(scl[:], rstd[:], std_sel[:, cs])
                tmp = stats.tile([128, 2], f32, tag="tmp")
                nc.vector.tensor_mul(tmp[:], mvp[:, 0:4:2], scl[:])
                shf = stats.tile([128, 2], f32, tag="shf")
                nc.vector.tensor_sub(shf[:], mean_sel[:, cs], tmp[:])
            affs.append((scl, shf))

        def emit_apply(ti, dve_half=False):
            x_sb, r0, _ = xts[ti]
            scl, shf = affs[ti // 2]
            c = ti % 2
            y_sb = ypool.tile([128, HW], f16, tag="yt")
            h0 = slice(0, HW // 2)
            h1 = slice(HW // 2, HW)
            with grp():
                if dve_half:
                    # drain mode: DVE (idle by now) takes one half, ScalarE
                    # the other; DVE's store rides the idle Sync ring
                    nc.vector.tensor_scalar(y_sb[:, h1], x_sb[:, h1],
                                            scl[:, c:c + 1], shf[:, c:c + 1],
                                            mybir.AluOpType.mult,
                                            mybir.AluOpType.add)
                    nc.sync.dma_start(y_d[r0:r0 + 128, h1], y_sb[:, h1])
                    nc.scalar.activation(
                        y_sb[:, h0], x_sb[:, h0],
                        mybir.ActivationFunctionType.Identity,
                        bias=shf[:, c:c + 1], scale=scl[:, c:c + 1])
                    nc.scalar.dma_start(y_d[r0:r0 + 128, h0], y_sb[:, h0])
                else:
                    for hs in (h0, h1):
                        nc.scalar.activation(
                            y_sb[:, hs], x_sb[:, hs],
                            mybir.ActivationFunctionType.Identity,
                            bias=shf[:, c:c + 1], scale=scl[:, c:c + 1])
                        nc.scalar.dma_start(y_d[r0:r0 + 128, hs], y_sb[:, hs])

        for p in range(BPC):
            t0, t1 = 2 * p, 2 * p + 1
            mvp = stats.tile([128, 4], f32, tag="mvp")
            g0 = emit_bn(t0, mvp, 0, nhead=NHEAD)
            next(g0)                      # BN head t0
            if p == 0:
                emit_protos()
            else:
                emit_chain(p - 1)
                emit_apply(2 * p - 2)
            for _ in g0:                  # BN tail t0 + aggr
                pass
            g1 = emit_bn(t1, mvp, 1, nhead=0)
            if p > 0:
                emit_apply(2 * p - 1)
            for _ in g1:                  # BN t1 + aggr
                pass
            emit_sqrt(p, mvp)
        emit_chain(BPC - 1)
        emit_apply(ntiles - 2, dve_half=True)
        emit_apply(ntiles - 1, dve_half=True)


def _program():
    if "nc" in _cache:
        return _cache["nc"]
    import concourse.bass as bass  # noqa: F401
    import concourse.tile as tile
    from concourse import bacc, mybir

    f32 = mybir.dt.float32
    f16 = mybir.dt.float16
    nc = bacc.Bacc("TRN2", target_bir_lowering=False, debug=False,
                   num_devices=NCORES)
    aps = [
        nc.dram_tensor("x", [ROWS, HW], f16, kind="ExternalInput").ap(),
        nc.dram_tensor("packed", [2 * K, PCOLS], f32, kind="ExternalInput").ap(),
        nc.dram_tensor("y", [ROWS, HW], f16, kind="ExternalOutput").ap(),
    ]
    with tile.TileContext(nc) as tc:
        _emit(tc, nc, mybir, aps)
    nc.compile()
    _cache["nc"] = nc
    return nc


def _run(inputs, trace=False, trace_cores=None):
    from concourse import bass_utils

    nc = _program()

    x = np.asarray(inputs["x"], dtype=np.float32)
    label = np.asarray(inputs["label"])
    w = np.asarray(inputs["combine_weights"], dtype=np.float32)
    pmp = np.ascontiguousarray(np.asarray(inputs["proto_mean_pos"], dtype=np.float32))
    psp = np.ascontiguousarray(np.asarray(inputs["proto_std_pos"], dtype=np.float32))
    pmn = np.ascontiguousarray(np.asarray(inputs["proto_mean_neg"], dtype=np.float32))
    psn = np.ascontiguousarray(np.asarray(inputs["proto_std_neg"], dtype=np.float32))

    is_pos = (label == 0).astype(np.float32)[:, None]   # [B,1]
    wpos = w * is_pos                                   # [B,K]
    wneg = w * (1.0 - is_pos)

    in_maps = []
    for c in range(NCORES):
        bs = slice(c * BPC, (c + 1) * BPC)
        packed = np.concatenate([
            np.concatenate([wpos[bs].T, wneg[bs].T], axis=0),
            np.concatenate([pmp, pmn], axis=0),
            np.concatenate([psp, psn], axis=0),
        ], axis=1)
        in_maps.append({
            "x": np.ascontiguousarray(x[bs]).reshape(ROWS, HW).astype(np.float16),
            "packed": np.ascontiguousarray(packed),
        })

    res = bass_utils.run_bass_kernel_spmd(
        nc, in_maps, core_ids=list(range(NCORES)),
        trace=trace, trace_cores=trace_cores,
    )
    out = np.concatenate(
        [np.asarray(res.results[c]["y"], dtype=np.float32).reshape(BPC, C, H, W)
         for c in range(NCORES)],
        axis=0,
    )
    return out, res


def kernel(**inputs):
    out, _ = _run(inputs, trace=False)
    return out



# revision 21
# speedup vs baseline: 1.4208x; 1.0064x over previous
"""Trainium2 Bass kernel for nn_CSA_36971078484033.

Instance-norm over (H,W) per (B,C) with a Dirichlet-weighted prototype affine
(label-conditional bank selection), data-parallel over B on 8 NeuronCores.

  out[b,c,h,w] = (x[b,c,h,w] - mean[b,c]) / sqrt(var[b,c] + eps) * new_std[b,c]
               + new_mean[b,c]
  new_mean = (label==0) ? w@proto_mean_pos : w@proto_mean_neg   (same for std)

Per core: 4 samples = 8 tiles of [128ch, 3136px].  Stats via bn_stats/bn_aggr
(DVE), affine apply via one ScalarE activation (out = x*scale + bias), the tiny
[64,4]x[64,256] prototype einsum on TensorE with the label selection folded
into host-masked weights (w*(label==0) and w*(label!=0) contribute to pos/neg
banks respectively; the unselected bank's weights are zero).

x/y travel as fp16 (host casts): per-core HBM traffic drops 25.7MB -> 12.8MB,
which is the binding roofline (~358 GB/s HBM per NC).  fp16 keeps 11 ktmantissa
bits: abs err ~5e-4 * |x|, orders below the 2e-2 gate.  Stats accumulate in
f32 inside DVE; ScalarE applies the f32 per-(b,c) affine with an fp16 cast on
the way out.
"""

import numpy as np
from contextlib import ExitStack

B, C, H, W = 32, 256, 56, 56
HW = H * W            # 3136
K = 64
EPS = 1e-5
NCORES = 8
BPC = B // NCORES     # 4 samples per core
ROWS = BPC * C        # 1024 DRAM rows per core
NCHUNK = 7
PCOLS = 4 + 2 * 256   # [wposT;wnegT] | [pmp;pmn] | [psp;psn], 128 rows
CHUNK = HW // NCHUNK  # 448 (<= bn_stats hw max of 512; equal chunks keep
                      # bn_aggr's equal-count variance combine exact)

_cache = {}


def _emit(tc, nc, mybir, aps):
    f32 = mybir.dt.float32
    f16 = mybir.dt.float16
    x_d, packed_d, y_d = aps
    with ExitStack() as ctx:
        consts = ctx.enter_context(tc.tile_pool(name="consts", bufs=1))
        xpool = ctx.enter_context(tc.tile_pool(name="xp", bufs=8))
        ypool = ctx.enter_context(tc.tile_pool(name="yp", bufs=4))
        stats = ctx.enter_context(tc.tile_pool(name="stats", bufs=4))
        psum = ctx.enter_context(tc.tile_pool(name="psum", bufs=2, space="PSUM"))

        # Scheduling: the Tile list-scheduler reorders per-engine streams
        # using its own sim; left alone it bunches the per-tile chain ops at
        # the end of the vector stream, which stalls ScalarE and serializes
        # a ~12us tail.  tile_wait_until(g) with a monotonically increasing
        # group index pins every engine's static order to exactly the
        # software pipeline below.
        gctr = [0]

        def grp(adv=True):
            w = tc.tile_wait_until(gctr[0])
            if adv:
                gctr[0] += 1
            return w

        # --- tiny inputs packed host-side into ONE [128, 516] tensor:
        # col 0:4   = [wposT; wnegT]  (label-masked Dirichlet weights, stacked
        #             pos-bank over neg-bank along the 128-partition dim)
        # col 4:260 = [pmp; pmn], col 260:516 = [psp; psn]
        # Dispatched on the Sync ring AHEAD of the x tiles (the Activation
        # ring starts with ~2.6us of ACT_TABLE_LOADs that would delay it).
        ntiles = BPC * 2
        NHEAD = 3
        xts = []
        packed_sb = consts.tile([2 * K, PCOLS], f32, tag="packed")
        eps_sb = consts.tile([128, 1], f32, tag="eps")
        with grp():
            nc.sync.dma_start(packed_sb[:], packed_d[:])
            nc.vector.memset(eps_sb[:], EPS)
            for ti in range(ntiles):
                b, h = divmod(ti, 2)
                r0 = b * C + h * 128
                x_sb = xpool.tile([128, HW], f16, tag="xt")
                if ti == 0:
                    # split tile 0 so bn_stats can start on the first NHEAD
                    # chunks ~1.5us before the whole tile lands
                    c0 = NHEAD * CHUNK
                    nc.sync.dma_start(x_sb[:, 0:c0], x_d[r0:r0 + 128, 0:c0])
                    nc.sync.dma_start(x_sb[:, c0:], x_d[r0:r0 + 128, c0:])
                else:
                    nc.sync.dma_start(x_sb[:], x_d[r0:r0 + 128, :])
                xts.append((x_sb, r0, h * BPC + b))
        w_sb = packed_sb[:, 0:BPC]
        pmean = packed_sb[:, BPC:BPC + C]
        pstd = packed_sb[:, BPC + C:BPC + 2 * C]
        mean_sel = consts.tile([128, 2 * BPC], f32, tag="mean_sel")
        std_sel = consts.tile([128, 2 * BPC], f32, tag="std_sel")

        def emit_protos():
            # selected new_mean/new_std, channel-major: [128ch, BPC] per
            # half; ONE 128-contraction matmul per (stat, chalf).  Runs
            # during the first x tile's in-DMA.
            with grp():
                for h in range(2):
                    cs = slice(h * 128, (h + 1) * 128)
                    bs = slice(h * BPC, (h + 1) * BPC)
                    pm = psum.tile([128, BPC], f32, tag="ps_mm")
                    nc.tensor.matmul(pm[:], pmean[:, cs], w_sb,
                                     start=True, stop=True)
                    nc.vector.tensor_copy(mean_sel[:, bs], pm[:])
                    ps = psum.tile([128, BPC], f32, tag="ps_ss")
                    nc.tensor.matmul(ps[:], pstd[:, cs], w_sb,
                                     start=True, stop=True)
                    nc.vector.tensor_copy(std_sel[:, bs], ps[:])

        # --- 8 tiles of [128, HW] processed as 4 PAIRS (tiles 2p, 2p+1 =
        # sample b, channel halves 0/1).  Stats per tile via bn_stats/
        # bn_aggr; the sqrt + affine chain is batched per pair as [128,2]
        # ops (strided APs over the pair's mv columns), which halves both
        # ScalarE's Sqrt<->Identity table-switch tax (~1.27us/switch-pair)
        # and the DVE small-op dispatch overhead (~160ns/op).
        # Steady-state emission per pair p (tiles t0=2p, t1=2p+1):
        #   [BN head t0] [chain p-1] [ID(2p-2)] [BN tail t0 + aggr]
        #   [BN head t1] [ID(2p-1)] [BN tail t1 + aggr] [sqrt pair p]
        mvps, affs = [], []

        def emit_bn(ti, mvp, mcol, nhead=0):
            x_sb, _, _ = xts[ti]
            st6 = stats.tile([128, NCHUNK * 6], f32, tag="st6")
            def run(lo, hi):
                for i in range(lo, hi):
                    nc.vector.bn_stats(st6[:, i * 6:(i + 1) * 6],
                                       x_sb[:, i * CHUNK:(i + 1) * CHUNK])
            if nhead:
                with grp():
                    run(0, nhead)
                yield
            with grp():
                run(nhead, NCHUNK)
                nc.vector.bn_aggr(mvp[:, 2 * mcol:2 * mcol + 2], st6[:])

        def emit_sqrt(p, mvp):
            # one Sqrt over both tiles' variances: [128,2] strided view
            stdv = stats.tile([128, 2], f32, tag="stdv")
            with grp():
                nc.scalar.activation(stdv[:], mvp[:, 1:4:2],
                                     mybir.ActivationFunctionType.Sqrt,
                                     bias=eps_sb[:],
                                     scale=float(HW) / float(HW - 1))
            mvps.append((mvp, stdv))

        def emit_chain(p):
            mvp, stdv = mvps[p]
            b = p  # pair p = sample b, cols b and BPC+b in *_sel
            cs = slice(b, 2 * BPC, BPC)
            with grp():
                rstd = stats.tile([128, 2], f32, tag="rstd")
                nc.vector.reciprocal(rstd[:], stdv[:])
                scl = stats.tile([128, 2], f32, tag="scl")
                nc.vector.tensor_mul(scl[:], rstd[:], std_sel[:, cs])
                tmp = stats.tile([128, 2], f32, tag="tmp")
                nc.vector.tensor_mul(tmp[:], mvp[:, 0:4:2], scl[:])
                shf = stats.tile([128, 2], f32, tag="shf")
                nc.vector.tensor_sub(shf[:], mean_sel[:, cs], tmp[:])
            affs.append((scl, shf))

        def emit_apply(ti, mode="scalar"):
            x_sb, r0, _ = xts[ti]
            scl, shf = affs[ti // 2]
            c = ti % 2
            y_sb = ypool.tile([128, HW], f16, tag="yt")
            h0 = slice(0, HW // 2)
            h1 = slice(HW // 2, HW)
            with grp():
                if mode == "dve":
                    # drain mode: DVE's fp16 2-ALU tensor_scalar runs the
                    # affine 2.4x faster than ScalarE; store rides the idle
                    # Sync ring
                    for hs in (h0, h1):
                        nc.vector.tensor_scalar(y_sb[:, hs], x_sb[:, hs],
                                                scl[:, c:c + 1],
                                                shf[:, c:c + 1],
                                                mybir.AluOpType.mult,
                                                mybir.AluOpType.add)
                        nc.sync.dma_start(y_d[r0:r0 + 128, hs], y_sb[:, hs])
                elif mode == "split":
                    nc.vector.tensor_scalar(y_sb[:, h1], x_sb[:, h1],
                                            scl[:, c:c + 1], shf[:, c:c + 1],
                                            mybir.AluOpType.mult,
                                            mybir.AluOpType.add)
                    nc.sync.dma_start(y_d[r0:r0 + 128, h1], y_sb[:, h1])
                    nc.scalar.activation(
                        y_sb[:, h0], x_sb[:, h0],
                        mybir.ActivationFunctionType.Identity,
                        bias=shf[:, c:c + 1], scale=scl[:, c:c + 1])
                    nc.scalar.dma_start(y_d[r0:r0 + 128, h0], y_sb[:, h0])
                else:
                    for hs in (h0, h1):
                        nc.scalar.activation(
                            y_sb[:, hs], x_sb[:, hs],
                            mybir.ActivationFunctionType.Identity,
                            bias=shf[:, c:c + 1], scale=scl[:, c:c + 1])
                        nc.scalar.dma_start(y_d[r0:r0 + 128, hs], y_sb[:, hs])

        for p in range(BPC):
            t0, t1 = 2 * p, 2 * p + 1
            mvp = stats.tile([128, 4], f32, tag="mvp")
            g0 = emit_bn(t0, mvp, 0, nhead=NHEAD)
            next(g0)                      # BN head t0
            if p == 0:
                emit_protos()
            else:
                emit_chain(p - 1)
                emit_apply(2 * p - 2)
            for _ in g0:                  # BN tail t0 + aggr
                pass
            g1 = emit_bn(t1, mvp, 1, nhead=0)
            if p > 0:
                emit_apply(2 * p - 1)
            for _ in g1:                  # BN t1 + aggr
                pass
            emit_sqrt(p, mvp)
        emit_chain(BPC - 1)
        emit_apply(ntiles - 2, mode="dve")
        emit_apply(ntiles - 1, mode="split")


def _program():
    if "nc" in _cache:
        return _cache["nc"]
    import concourse.bass as bass  # noqa: F401
    import concourse.tile as tile
    from concourse import bacc, mybir

    f32 = mybir.dt.float32
    f16 = mybir.dt.float16
    nc = bacc.Bacc("TRN2", target_bir_lowering=False, debug=False,
                   num_devices=NCORES)
    aps = [
        nc.dram_tensor("x", [ROWS, HW], f16, kind="ExternalInput").ap(),
        nc.dram_tensor("packed", [2 * K, PCOLS], f32, kind="ExternalInput").ap(),
        nc.dram_tensor("y", [ROWS, HW], f16, kind="ExternalOutput").ap(),
    ]
    with tile.TileContext(nc) as tc:
        _emit(tc, nc, mybir, aps)
    nc.compile()
    _cache["nc"] = nc
    return nc


def _run(inputs, trace=False, trace_cores=None):
    from concourse import bass_utils

    nc = _program()

    x = np.asarray(inputs["x"], dtype=np.float32)
    label = np.asarray(inputs["label"])
    w = np.asarray(inputs["combine_weights"], dtype=np.float32)
    pmp = np.ascontiguousarray(np.asarray(inputs["proto_mean_pos"], dtype=np.float32))
    psp = np.ascontiguousarray(np.asarray(inputs["proto_std_pos"], dtype=np.float32))
    pmn = np.ascontiguousarray(np.asarray(inputs["proto_mean_neg"], dtype=np.float32))
    psn = np.ascontiguousarray(np.asarray(inputs["proto_std_neg"], dtype=np.float32))

    is_pos = (label == 0).astype(np.float32)[:, None]   # [B,1]
    wpos = w * is_pos                                   # [B,K]
    wneg = w * (1.0 - is_pos)

    in_maps = []
    for c in range(NCORES):
        bs = slice(c * BPC, (c + 1) * BPC)
        packed = np.concatenate([
            np.concatenate([wpos[bs].T, wneg[bs].T], axis=0),
            np.concatenate([pmp, pmn], axis=0),
            np.concatenate([psp, psn], axis=0),
        ], axis=1)
        in_maps.append({
            "x": np.ascontiguousarray(x[bs]).reshape(ROWS, HW).astype(np.float16),
            "packed": np.ascontiguousarray(packed),
        })

    res = bass_utils.run_bass_kernel_spmd(
        nc, in_maps, core_ids=list(range(NCORES)),
        trace=trace, trace_cores=trace_cores,
    )
    out = np.concatenate(
        [np.asarray(res.results[c]["y"], dtype=np.float32).reshape(BPC, C, H, W)
         for c in range(NCORES)],
        axis=0,
    )
    return out, res


def kernel(**inputs):
    out, _ = _run(inputs, trace=False)
    return out

